# revision 4
# baseline (speedup 1.0000x reference)
"""Distributed inverse real SHT on 8 Trainium2 NeuronCores (Bass/Tile).

Math (per reference):
    S[c,k,m]  = sum_l x[c,m,l] * pct[m,k,l]          (Legendre synthesis)
    y[c,k,n]  = irfft_{n=1024}(S, norm='forward')
              = sum_m  Sre[c,k,m]*Fc[m,n] + Sim[c,k,m]*Fs[m,n]
    with Fc[m,n] = w_m cos(2*pi*m*n/N), Fs[m,n] = -w_m sin(2*pi*m*n/N),
    w_0 = 1, w_m = 2 otherwise (verified exactly vs np.fft.irfft).
    pct[m,*,l] = 0 for l < m (triangular), and the m=512 row of pct is
    entirely zero (l < 512 always), so the effective mmax is 512.

Sharding: nlat (k) split across the 8 cores -> 64 output latitudes per
core, no inter-core communication.  Each core streams a packed (l >= m)
fp16 slice of pct and x (two-piece DMAs skip the zero rows of short
tiles), does per-m-pair matmuls into PSUM, and per 128-m block stages
S^T, PE-transposes per channel pair, applies the DFT matmuls, and
accumulates into an fp16 SBUF accumulator.  Elementwise work is spread
across DVE / ACT (PSUM-capable) and GpSimd (SBUF-only) so no single
engine exceeds the DMA streaming window.
"""

import numpy as np
from contextlib import ExitStack


NLAT, NLON = 512, 1024
LMAX, MMAX = 512, 513
M_E = 512            # effective mmax (m=512 row of pct is identically zero)
B, C = 1, 16
NCORES = 8
KC = NLAT // NCORES  # 64 latitudes per core
PAIRS = M_E // 2     # 256 m-pairs
TILE_W = 192         # 128 pct cols (2m x 64k) + 64 x cols (2m x 2ri x 16c)
NBLK = 4             # 128-m blocks


def _plan():
    """One column (192 f-cols x <=128 rows) per (pair,chunk) tile, sorted by
    K descending within each PSUM bank so the slab DMA can be split in two
    pieces: full-height for the leading K>64 columns, trimmed height for
    the rest (the skipped rows are identically zero).

    Returns (bank_ops, slab_info, total_cols); bank_ops[G] is a list of
    (pair, l0, K, col) with col the GLOBAL column index; slab_info[G] is
    (w, w64, h2): total columns, columns needing rows beyond 64, and the
    max K among the trailing w-w64 columns."""
    bank_ops = []
    slab_info = []
    ncols = 0
    for G in range(PAIRS // 8):
        tiles = []
        for t in range(8 * G, 8 * G + 8):
            l0 = 2 * t
            L = LMAX - l0
            nch = (L + 127) // 128
            for c in range(nch):
                tiles.append((t, l0 + 128 * c, min(128, L - 128 * c)))
        tiles.sort(key=lambda x: -x[2])
        ops = [(t, l0, K, ncols + i) for i, (t, l0, K) in enumerate(tiles)]
        bank_ops.append(ops)
        w = len(tiles)
        w64 = sum(1 for (_, _, K) in tiles if K > 64)
        h2 = max([K for (_, _, K) in tiles if K <= 64], default=0)
        slab_info.append((w, w64, h2))
        ncols += w
    return bank_ops, slab_info, ncols


_BANK_OPS, _SLAB_INFO, NCOLS = _plan()
_SLAB_COL0 = np.cumsum([0] + [w for (w, _, _) in _SLAB_INFO])
F_TOT = NCOLS * TILE_W

# Even/odd DFT folding: compute E[n'] = sum_m wc*Re and O[n'] = sum_m ws*Im
# for n' in [0,512) plus the y[512] column (folded into O's zero column 0);
# then y[n'] = E+O, y[1024-n'] = E-O.
NE = NLON // 2       # 512
FW = NE + NE + 16    # wc | ws | (y512 col + pad)


def build_program():
    from concourse import bacc, bass, masks, mybir, tile

    dt = mybir.dt
    nc = bacc.Bacc("TRN2", target_bir_lowering=False, debug=False,
                   num_devices=NCORES)

    stream = nc.dram_tensor("stream", [128, F_TOT], dt.float16,
                            kind="ExternalInput")
    fmat = nc.dram_tensor("fmat", [128, NBLK * FW], dt.float16,
                          kind="ExternalInput")
    y = nc.dram_tensor("y", [C * KC, NLON], dt.float16, kind="ExternalOutput")

    with tile.TileContext(nc) as tc, ExitStack() as ctx:
        sp = ctx.enter_context(tc.tile_pool(name="stream", bufs=8))
        cp = ctx.enter_context(tc.tile_pool(name="const", bufs=1))
        fp = ctx.enter_context(tc.tile_pool(name="fsb", bufs=NBLK))
        ysp = ctx.enter_context(tc.tile_pool(name="ysb", bufs=3))
        snp = ctx.enter_context(tc.tile_pool(name="snat", bufs=2))
        lhp = ctx.enter_context(tc.tile_pool(name="lhs", bufs=3))
        tmp = ctx.enter_context(tc.tile_pool(name="tmp16", bufs=3))
        ps1 = ctx.enter_context(
            tc.tile_pool(name="ps1", bufs=2, space=bass.MemorySpace.PSUM))
        pst = ctx.enter_context(
            tc.tile_pool(name="pst", bufs=2, space=bass.MemorySpace.PSUM))
        ps2 = ctx.enter_context(
            tc.tile_pool(name="ps2", bufs=2, space=bass.MemorySpace.PSUM))

        # fp16 output accumulator: partition = (c2,k64) within strip,
        # free = strip*1024 + n  (E in [0:512), O in [512:1024))
        acc = cp.tile([128, 8 * NLON], dt.float16)

        slabs = {}

        def get_slab(si):
            si = min(si, len(_SLAB_INFO) - 1)
            if si not in slabs:
                w, w64, h2 = _SLAB_INFO[si]
                st = sp.tile([128, w * TILE_W], dt.float16, tag="slab")
                o0 = int(_SLAB_COL0[si]) * TILE_W
                if w64:
                    nc.sync.dma_start(
                        out=st[:, 0:w64 * TILE_W],
                        in_=stream[:, o0:o0 + w64 * TILE_W])
                if w64 < w:
                    nc.sync.dma_start(
                        out=st[0:h2, w64 * TILE_W:w * TILE_W],
                        in_=stream[0:h2,
                                   o0 + w64 * TILE_W:o0 + w * TILE_W])
                slabs[si] = st
            return slabs[si]

        # Process m-blocks smallest-first (block 3 has the shortest pairs,
        # block 0 the longest): each block's deferred transpose+DFT work
        # then hides inside the NEXT block's larger DMA window.
        border = list(range(NBLK - 1, -1, -1))
        bank_seq = [b * 8 + g for b in border for g in range(8)]
        seq_pos = {G: i for i, G in enumerate(bank_seq)}

        # prefetch the first banks in processing order, then the (tiny)
        # DFT matrices for all four blocks
        get_slab(bank_seq[0])
        get_slab(bank_seq[1])
        fsbs = {}
        for b in border:
            fsbs[b] = fp.tile([128, FW], dt.float16, tag="fsb",
                              name=f"fsb{b}")
            nc.sync.dma_start(
                out=fsbs[b][:], in_=fmat[:, b * FW:(b + 1) * FW])
        get_slab(bank_seq[2])

        ident = cp.tile([128, 128], dt.float16)
        masks.make_identity(nc, ident[:])

        deferred = []  # previous block's transpose + DFT work, as thunks

        def make_strip_thunk(bi, b, s8, snat_m, fsb):
            """Strip s8's post-stage-1 work for block b: two PE transposes
            into a per-strip lhsT (via one PSUM->SBUF copy), the DFT
            matmuls, and the fp16 SBUF accumulation; on the final block
            also the E/O fold and the output DMAs."""
            def thunk():
                pt = pst.tile([128, 256], dt.float16, tag="pt")
                nc.tensor.transpose(pt[:, 0:128], snat_m[:, :, 2 * s8],
                                    ident[:])
                nc.tensor.transpose(pt[:, 128:256],
                                    snat_m[:, :, 2 * s8 + 1], ident[:])
                # pt free = (c2, ri2, k64) -> lhs free = (ri2, c2, k64)
                lhs = lhp.tile([128, 256], dt.float16, tag="lhs")
                pt_v = pt[:].rearrange("p (c r k) -> p c r k", c=2, r=2)
                lhs_v = lhs[:].rearrange("p (r c k) -> p r c k", r=2, c=2)
                ceng = nc.vector.tensor_copy if s8 % 2 == 0 else nc.scalar.copy
                ceng(lhs_v[:, :, 0, :], pt_v[:, 0, :, :])
                ceng(lhs_v[:, :, 1, :], pt_v[:, 1, :, :])

                yp = ps2.tile([128, 1024], dt.float32, tag="yp")
                l0 = lhs[:, 0:128]       # Re rows
                l1 = lhs[:, 128:256]     # Im rows
                # E (bank A), O + y512-column (bank B)
                nc.tensor.matmul(yp[:, 0:NE], l0, fsb[:, 0:NE],
                                 start=True, stop=True)
                nc.tensor.matmul(yp[:, NE:2 * NE], l1, fsb[:, NE:2 * NE],
                                 start=True, stop=False)
                nc.tensor.matmul(yp[:, NE:NE + 8], l0,
                                 fsb[:, 2 * NE:2 * NE + 8],
                                 start=False, stop=True)

                a_sl = acc[:, s8 * NLON:(s8 + 1) * NLON]
                if bi == 0:
                    ieng = (nc.vector.tensor_copy if s8 % 2 == 0
                            else nc.scalar.copy)
                    ieng(a_sl, yp[:])
                else:
                    t16 = tmp.tile([128, NLON], dt.float16, tag="t16")
                    peng = (nc.scalar.copy if s8 % 2 == 0
                            else nc.vector.tensor_copy)
                    peng(t16[:], yp[:])
                    aeng = (nc.gpsimd.tensor_add if s8 % 2 == 0
                            else nc.vector.tensor_add)
                    aeng(a_sl, a_sl, t16[:])
                if bi == NBLK - 1:
                    # fold E/O into the full spectrum and write out.  All
                    # operands fp16 SBUF: DVE runs the E-half add in 2x
                    # mode, GpSimd does the mirrored-half subtract.
                    base = s8 * NLON
                    ysb = ysp.tile([128, NLON], dt.float16, tag="ysb")
                    nc.vector.tensor_add(
                        ysb[:, 1:NE],
                        acc[:, base + 1:base + NE],
                        acc[:, base + NE + 1:base + 2 * NE])
                    nc.scalar.copy(ysb[:, 0:1], acc[:, base:base + 1])
                    nc.sync.dma_start(
                        out=y[s8 * 128:(s8 + 1) * 128, 0:NE],
                        in_=ysb[:, 0:NE])
                    nc.scalar.copy(
                        ysb[:, NE:NE + 1], acc[:, base + NE:base + NE + 1])
                    nc.gpsimd.tensor_sub(
                        ysb[:, NE + 1:NLON],
                        acc[:, base + NE - 1:base:-1],
                        acc[:, base + 2 * NE - 1:base + NE:-1])
                    nc.sync.dma_start(
                        out=y[s8 * 128:(s8 + 1) * 128, NE:NLON],
                        in_=ysb[:, NE:NLON])
            return thunk

        for bi, b in enumerate(border):
            fsb = fsbs[b]

            # S^T staging for this 128-m block:
            #   partition = ri*64+k, free = m_loc*16 + c   (fp16)
            snat = snp.tile([128, 128 * C], dt.float16, tag="snat")
            snat_g = snat[:].rearrange("p (g s two c) -> p g s two c",
                                       g=8, s=8, two=2, c=C)
            snat_m = snat[:].rearrange("p (m c) -> p m c", c=C)

            # ---- stage 1: Legendre matmuls, 8 m-pairs per PSUM bank ----
            for g in range(8):
                G = b * 8 + g
                st = get_slab(G)
                pos = seq_pos[G]
                if pos + 1 < len(bank_seq):
                    get_slab(bank_seq[pos + 1])
                if pos + 2 < len(bank_seq):
                    get_slab(bank_seq[pos + 2])
                # drain a deferred strip from the previous block FIRST:
                # its inputs are long ready, so the in-order PE fills the
                # wait for this bank's slab with useful transpose/DFT work
                if deferred:
                    deferred.pop(0)()
                pb = ps1.tile([128, 512], dt.float32, tag="pb")
                pb_v = pb[:].rearrange("p (s mj r c) -> p s mj r c",
                                       s=8, mj=2, r=2, c=C)
                ops = _BANK_OPS[G]
                for j, (t, l0, K, col) in enumerate(ops):
                    o = (col - int(_SLAB_COL0[G])) * TILE_W
                    s = t % 8
                    nc.tensor.matmul(
                        pb[:, s * 64:(s + 1) * 64],
                        st[0:K, o:o + 128],        # (K x [2m x 64k])
                        st[0:K, o + 128:o + 192],  # (K x [2m,2ri,16c])
                        start=(j == 0), stop=(j == len(ops) - 1))
                # extract diagonal (mi==mj) blocks -> snat (cast fp16),
                # split across DVE and ACT
                for mi in range(2):
                    for r in range(2):
                        eng = (nc.vector.tensor_copy if r == 0
                               else nc.scalar.copy)
                        eng(snat_g[r * 64:(r + 1) * 64, g, :, mi, :],
                            pb_v[mi * 64:(mi + 1) * 64, :, mi, r, :])

            while deferred:
                deferred.pop(0)()
            deferred = [make_strip_thunk(bi, b, s8, snat_m, fsb)
                        for s8 in range(8)]

        # last block's work has no next block to hide in
        while deferred:
            deferred.pop(0)()

    nc.compile()
    return nc


def _build_fmat():
    m = np.arange(M_E)
    n2 = np.arange(NE)
    w = np.where(m == 0, 1.0, 2.0)
    ang = 2.0 * np.pi * np.outer(m, n2) / NLON
    wc = (w[:, None] * np.cos(ang)).astype(np.float16)     # E weights
    ws = (-w[:, None] * np.sin(ang)).astype(np.float16)    # O weights
    fz = (w * np.where(m % 2 == 0, 1.0, -1.0)).astype(np.float16)  # y[512]
    fmat = np.zeros((128, NBLK * FW), np.float16)
    for b in range(NBLK):
        sl = slice(b * 128, (b + 1) * 128)
        fmat[:, b * FW:b * FW + NE] = wc[sl]
        fmat[:, b * FW + NE:b * FW + 2 * NE] = ws[sl]
        fmat[:, b * FW + 2 * NE] = fz[sl]
    return fmat


_ALL_OPS = [op for ops in _BANK_OPS for op in ops]


def _pack_streams(x_re, x_im, pct):
    """Per-core packed fp16 stream of shelf-packed (<=128 x 192) tiles."""
    x_re = np.asarray(x_re, np.float32)
    x_im = np.asarray(x_im, np.float32)
    pct = np.asarray(pct, np.float32)

    # x part is core-independent: build once
    template = np.zeros((128, F_TOT), np.float16)
    tv = template.reshape(128, NCOLS, TILE_W)
    for (t, l0, K, col) in _ALL_OPS:
        xr = x_re[0, :, l0:l0 + K, 2 * t:2 * t + 2]   # (c, K, 2m)
        xi = x_im[0, :, l0:l0 + K, 2 * t:2 * t + 2]
        xx = np.stack([xr, xi], axis=0)                # (r, c, K, m)
        tv[0:K, col, 128:] = xx.transpose(2, 3, 0, 1).reshape(K, 64)

    streams = []
    for core in range(NCORES):
        k0 = core * KC
        sbuf = template.copy()
        sv = sbuf.reshape(128, NCOLS, TILE_W)
        for (t, l0, K, col) in _ALL_OPS:
            blk = pct[2 * t:2 * t + 2, k0:k0 + KC, l0:l0 + K]  # (2m, 64k, K)
            sv[0:K, col, 0:128] = blk.transpose(2, 0, 1).reshape(K, 128)
        streams.append(sbuf)
    return streams


_NC_CACHE = [None]


def _get_program():
    if _NC_CACHE[0] is None:
        _NC_CACHE[0] = build_program()
    return _NC_CACHE[0]


def run(x_re, x_im, pct, nlon=NLON, trace=False, trace_kwargs=None):
    from concourse.bass_utils import run_bass_kernel_spmd

    assert int(nlon) == NLON
    nc = _get_program()
    fmat = _build_fmat()
    streams = _pack_streams(x_re, x_im, pct)
    in_maps = [{"stream": streams[i], "fmat": fmat} for i in range(NCORES)]
    res = run_bass_kernel_spmd(nc, in_maps, list(range(NCORES)),
                               trace=trace, **(trace_kwargs or {}))
    out = np.empty((B, C, NLAT, NLON), np.float32)
    for core in range(NCORES):
        yc = res.results[core]["y"].astype(np.float32).reshape(C, KC, NLON)
        out[0, :, core * KC:(core + 1) * KC, :] = yc
    return out, res


def kernel(x_re, x_im, pct, nlon=NLON, **_unused):
    out, _ = run(x_re, x_im, pct, nlon)
    return out
